# revision 22
# baseline (speedup 1.0000x reference)
"""Trainium2 Bass kernel for nn_BCE_Loss (focal-style BCE-with-logits, mean).

Reference math per anchor row x[0:3] (logits) and integer target c:
    col = 0 if c==1 else 1 if c==3 else 2
    t   = one_hot(col, 3)
    w   = (1-pt)^2,  pt = x*t + (1-x)*(1-t)        [from detached logits]
    bce = max(x,0) - x*t + log1p(exp(-|x|))
    out = mean(w * bce)

Identity used on device (exact, per element, t in {0,1}):
    loss = u^2 * sp,   u = v + t,  v = x*(1-2t),  sp = softplus(v)

With g = 0.5 - t (so v = 2*g*x) and h = g*x, expand u^2 using the mask
identities g^2 = 1/4, h*g = x/4, h^2 = x^2/4:
    u^2 = (2h + 0.5 - g)^2 = x^2 + 2h - g - x + 0.5
    sum loss = sum x^2*sp + 2*sum h*sp - sum g*sp - sum x*sp + 0.5*sum sp

All four pair-sums share the SAME right factor sp, so the four left
operands are stored as contiguous planes [xsq | h | g | x] of one tile and
each 128-wide chunk needs ONE TensorE matmul: stationary = sp chunk,
moving = the four planes (512 wide, one PSUM bank). The [1, 2, -1, -1]
coefficients ride the host-built block-diagonal extraction constant, and
0.5*sum sp rides the Ln op's accum_out for free. This removes the
u/u^2/w ops entirely: no ACT Square pass, Vector does only 5 ops/chunk.

Engine split per chunk (elementwise tensors bf16, dense step-1):
    DVE: one-hot g from int32 targets (3 strided 1x ops into the g plane),
         h = g*x (2x), xsq = x*x (2x)
    ACT: E = Exp(2h); sp = Ln(E+1) in place, accum_out = row-sums of sp
         (both live in the pinned natural_log_exp_and_others set)
    PE:  psA[:, q*128+j] += sp_c^T @ plane_q chunk  (one matmul per chunk)
    GPSIMD: the SWDGE cast-DMA loading x f32->bf16 into its plane
First/last chunks are half-size to shorten pipeline fill and drain.

Sharding: pure data-parallel across 8 NeuronCores - each core takes a
contiguous block of anchors; per-core output is a single partial sum; the
host sums the 8 partials and divides by the element count.
"""

import numpy as np

import concourse.bacc as bacc
import concourse.bass as bass
import concourse.mybir as mybir
from concourse import bass_utils
from concourse.alu_op_type import AluOpType
from concourse.tile import TileContext

N_CORES = 8
N_ANCHORS = 8388608
N_CLASSES = 3
N_SHARD = N_ANCHORS // N_CORES  # 1048576
P = 128  # SBUF partitions
T = 1024  # anchor rows per partition per DRAM-view tile
NT = N_SHARD // (P * T)  # 8 tiles per core
F = N_CLASSES * T  # free dim of an x tile
MM = 128  # diag-trick matmul chunk width
NQ = 4  # planes: xsq, h, g, x
COEF = (1.0, 2.0, -1.0, -1.0)

# (tile, row_lo, row_hi): half-size edge chunks shorten pipeline fill/drain
CHUNKS = (
    [(0, 0, 512), (0, 512, 1024)]
    + [(i, 0, 1024) for i in range(1, NT - 1)]
    + [(NT - 1, 0, 512), (NT - 1, 512, 1024)]
)
# chunks whose x^2 runs on ACT (Square) instead of DVE, balancing the engines
XSQ_ON_ACT = ()


class _Bacc(bacc.Bacc):
    """Bacc with the ACT table pinned to natural_log_exp_and_others.

    The default chooser puts Exp in exp_and_others and Ln in natural_log,
    reloading tables every tile (~1.3us each). Both live in
    natural_log_exp_and_others; emptying every other set (positions kept -
    act_func_set_id is the index into act_info.json) forces one load."""

    _ACT_SET = "natural_log_exp_and_others"

    def insert_act_table_loads(self):
        import bass_rust as _bass_rust

        from concourse.hw_specs import get_activation_tables

        has_activation = any(
            isinstance(i, mybir.InstActivation)
            for b in self.main_func.blocks
            for i in b.instructions
        )
        if not has_activation:
            return
        tables = [
            (name, (fns if name == self._ACT_SET else set()))
            for name, fns in get_activation_tables(self.m.arch).items()
        ]
        _bass_rust.insert_act_table_loads(self, tables)


def _build_nc(targ_is_int64: bool) -> bass.Bass:
    nc = _Bacc("TRN2", target_bir_lowering=False, num_swdge_queues=4)
    pred = nc.dram_tensor(
        "pred", [N_SHARD, N_CLASSES], mybir.dt.float32, kind="ExternalInput"
    )
    n_targ_words = N_SHARD * (2 if targ_is_int64 else 1)
    targ = nc.dram_tensor("targ32", [n_targ_words], mybir.dt.int32, kind="ExternalInput")
    ident = nc.dram_tensor("id4", [P, NQ * MM], mybir.dt.bfloat16, kind="ExternalInput")
    out = nc.dram_tensor("out", [1], mybir.dt.float32, kind="ExternalOutput")

    xv = pred.rearrange("(n p t) m -> n p (t m)", p=P, t=T)
    tw = 2 * T if targ_is_int64 else T
    tv = targ.rearrange("(n p t) -> n p t", p=P, t=tw)

    nchunks = len(CHUNKS)
    with TileContext(nc) as tc:
        with (
            tc.tile_pool(name="zp", bufs=4) as zp,
            tc.tile_pool(name="io", bufs=4) as io,
            tc.tile_pool(name="ep", bufs=4) as ep,
            tc.tile_pool(name="singles", bufs=1) as singles,
            tc.tile_pool(name="psum", bufs=1, space="PSUM") as psum,
        ):
            ones_f = singles.tile([P, 1], mybir.dt.float32)
            nc.vector.memset(ones_f, 1.0)
            accsp = singles.tile([P, nchunks], mybir.dt.float32)
            nc.vector.memset(accsp, 0.0)
            psA = psum.tile([P, NQ * MM], mybir.dt.float32)

            for ci, (i, r0, r1) in enumerate(CHUNKS):
                fc = (r1 - r0) * N_CLASSES
                x_ap = xv[i][:, r0 * N_CLASSES : r1 * N_CLASSES]
                wmul = 2 if targ_is_int64 else 1
                tg = io.tile([P, (r1 - r0) * wmul], mybir.dt.int32)
                nc.sync.dma_start(out=tg, in_=tv[i][:, r0 * wmul : r1 * wmul])
                if targ_is_int64:
                    tlo = tg.rearrange("p (t two) -> p t two", two=2)[:, :, 0]
                else:
                    tlo = tg

                # planes of Z: 0 = xsq, 1 = h, 2 = g, 3 = x
                Z = zp.tile([P, NQ * fc], mybir.dt.bfloat16)
                Zq = Z.rearrange("p (q f) -> p q f", q=NQ)
                xsq = Zq[:, 0, :]
                h = Zq[:, 1, :]
                g = Zq[:, 2, :]
                xb = Zq[:, 3, :]

                # x loaded with f32->bf16 cast in the DMA datapath (SWDGE)
                nc.gpsimd.dma_start(out=xb, in_=x_ap)

                # g = 0.5 - t (expanded one-hot), strided per-class writes
                g3 = g.rearrange("p (t m) -> p t m", m=N_CLASSES)
                nc.vector.tensor_scalar(
                    out=g3[:, :, 0], in0=tlo, scalar1=1, scalar2=0.5,
                    op0=AluOpType.not_equal, op1=AluOpType.subtract)
                nc.vector.tensor_scalar(
                    out=g3[:, :, 1], in0=tlo, scalar1=3, scalar2=0.5,
                    op0=AluOpType.not_equal, op1=AluOpType.subtract)
                # g2 = 0.5 - t2 = (0.5 - g0) - g1  (reverse0: scalar - in0)
                ig2 = nc.vector.scalar_tensor_tensor(
                    out=g3[:, :, 2], in0=g3[:, :, 0], scalar=0.5, in1=g3[:, :, 1],
                    op0=AluOpType.subtract, op1=AluOpType.subtract)
                ig2.ins.reverse0 = True

                # h = g * x  (v = 2h) ; xsq = x^2
                nc.vector.tensor_tensor(out=h, in0=g, in1=xb, op=AluOpType.mult)
                if ci in XSQ_ON_ACT:
                    nc.scalar.activation(
                        out=xsq, in_=xb, func=mybir.ActivationFunctionType.Square)
                else:
                    nc.vector.tensor_tensor(out=xsq, in0=xb, in1=xb, op=AluOpType.mult)

                # E = exp(2h) = e^v ; sp = ln(E+1) = softplus(v), in place;
                # accum_out picks up row-sums of sp for the 0.5*sum sp term
                sp = ep.tile([P, fc], mybir.dt.bfloat16)
                nc.scalar.activation(
                    out=sp, in_=h, func=mybir.ActivationFunctionType.Exp, scale=2.0)
                nc.scalar.activation(
                    out=sp, in_=sp, func=mybir.ActivationFunctionType.Ln, bias=1.0,
                    accum_out=accsp[:, ci : ci + 1])

                # TensorE: one matmul per 128-chunk covers all four pair-sums
                for c in range(fc // MM):
                    s = slice(c * MM, (c + 1) * MM)
                    nc.tensor.matmul(
                        psA[:, :], sp[:, s], Zq[:, :, s],
                        start=(ci == 0 and c == 0),
                        stop=(ci == nchunks - 1 and c == fc // MM - 1))

            # epilogue: total = sum(psA o id4) + 0.5 * sum(accsp)
            id_t = singles.tile([P, NQ * MM], mybir.dt.bfloat16)
            nc.sync.dma_start(out=id_t, in_=ident[:, :])
            dA = singles.tile([P, NQ * MM], mybir.dt.float32)
            nc.vector.tensor_tensor(out=dA, in0=psA, in1=id_t, op=AluOpType.mult)
            rA = singles.tile([P, 1], mybir.dt.float32)
            nc.vector.tensor_reduce(
                out=rA, in_=dA, axis=mybir.AxisListType.X, op=AluOpType.add)
            rS = singles.tile([P, 1], mybir.dt.float32)
            nc.vector.tensor_reduce(
                out=rS, in_=accsp, axis=mybir.AxisListType.X, op=AluOpType.add)
            rC = singles.tile([P, 1], mybir.dt.float32)
            nc.vector.scalar_tensor_tensor(
                out=rC, in0=rS, scalar=0.5, in1=rA,
                op0=AluOpType.mult, op1=AluOpType.add)

            psT = psum.tile([1, 1], mybir.dt.float32)
            nc.tensor.matmul(psT[:, :], ones_f[:, :], rC[:, :], start=True, stop=True)
            res = singles.tile([1, 1], mybir.dt.float32)
            nc.vector.tensor_copy(out=res, in_=psT)
            nc.sync.dma_start(out=out[:], in_=res[0, :])

    nc.compile()
    return nc


_cache: dict[bool, bass.Bass] = {}
last_results = None  # BassKernelResults of the most recent run (for test.py)


def _get_nc(targ_is_int64: bool) -> bass.Bass:
    if targ_is_int64 not in _cache:
        _cache[targ_is_int64] = _build_nc(targ_is_int64)
    return _cache[targ_is_int64]


def _id4_bf16() -> np.ndarray:
    import ml_dtypes

    eye = np.eye(P, dtype=np.float32)
    blocks = [c * eye for c in COEF]
    return np.concatenate(blocks, axis=1).astype(ml_dtypes.bfloat16)


def kernel(pred: np.ndarray, targ: np.ndarray, *, trace: bool = False) -> np.ndarray:
    global last_results
    pred = np.ascontiguousarray(np.asarray(pred, dtype=np.float32))
    targ = np.asarray(targ)
    assert pred.shape == (N_ANCHORS, N_CLASSES), pred.shape
    assert targ.shape == (N_ANCHORS,), targ.shape

    targ_is_int64 = targ.dtype.itemsize == 8
    if targ_is_int64:
        targ_words = np.ascontiguousarray(targ).view(np.int32)  # [2*N] lo,hi pairs
        words_per_shard = 2 * N_SHARD
    else:
        targ_words = np.ascontiguousarray(targ.astype(np.int32, copy=False))
        words_per_shard = N_SHARD

    nc = _get_nc(targ_is_int64)
    id4 = _id4_bf16()

    in_maps = []
    for c in range(N_CORES):
        in_maps.append({
            "pred": pred[c * N_SHARD : (c + 1) * N_SHARD],
            "targ32": targ_words[c * words_per_shard : (c + 1) * words_per_shard],
            "id4": id4,
        })

    res = bass_utils.run_bass_kernel_spmd(
        nc, in_maps, core_ids=list(range(N_CORES)), trace=trace
    )
    last_results = res

    total = np.float64(0.0)
    for r in res.results:
        total += np.float64(r["out"][0])
    mean = total / (N_ANCHORS * N_CLASSES)
    return np.float32(mean)
